# revision 7
# baseline (speedup 1.0000x reference)
"""Distributed causal multi-head attention block for 8 TRN2 NeuronCores.

Sharding: core i -> (batch b = i//2, head-half hh = i%2).  Each core computes
attention for 6 of the 12 heads of one batch element, then a row-sharded
c_proj (its 384 input channels -> full 768 outputs, partial sums).  The
host sums the two partial projections per batch (the "all-reduce" of the
tensor-parallel c_proj) and adds b_proj.

Everything on-chip lives transposed ([feature, token]) so no transposes are
needed:
  qkT = W_qk @ x^T          (heads' Q^T,K^T in [d, t] layout)
  V   = x @ Wv^T            ([t, d] layout, + per-head ones column)
  S^T = K_blk @ Q_blk^T     -> exp (scale 1/8 folded in) -> causal mask
  O^T_aug = [V|1]^T @ P^T   (row 64 of each head block = softmax denom)
  y^T = O^T * (1/denom)  + bv
  out^T = Wp_half @ y^T     (partial over this core's heads)
"""

import sys

sys.path.insert(0, "/opt/trn_rl_repo")

import numpy as np
import ml_dtypes

import concourse.bass as bass
import concourse.bacc as bacc
import concourse.mybir as mybir
import concourse.tile as tile
from concourse.bass_utils import run_bass_kernel_spmd

BF16 = mybir.dt.bfloat16
F32 = mybir.dt.float32
F32R = mybir.dt.float32r
AF = mybir.ActivationFunctionType
ALU = mybir.AluOpType

B, T, C, H, HD = 4, 2048, 768, 12, 64
NCORES = 8
HH = 6              # heads per core
CH = HH * HD        # 384 channels per core
NCT = C // 128      # 6 contraction tiles over C
NTT = T // 128      # 16 token tiles
NQC = T // 512      # 4 query chunks
VW = 65             # per-head V block width (64 dims + ones column)


def _build_graph():
    nc = bacc.Bacc("TRN2", target_bir_lowering=False)

    xT = nc.declare_dram_parameter("xT", [C, T], BF16, isOutput=False)
    wqkT = nc.declare_dram_parameter("wqkT", [C, 2 * CH], BF16, isOutput=False)
    bqk = nc.declare_dram_parameter("bqk", [128, 2 * CH // 128], F32, isOutput=False)
    wvT = nc.declare_dram_parameter("wvT", [C, CH], BF16, isOutput=False)
    bv = nc.declare_dram_parameter("bv", [128, CH // 128], F32, isOutput=False)
    wpT = nc.declare_dram_parameter("wpT", [CH, C], BF16, isOutput=False)
    masks = nc.declare_dram_parameter("masks", [128, 4 * 512], BF16, isOutput=False)
    out = nc.declare_dram_parameter("out", [C, T], F32, isOutput=True)

    with tile.TileContext(nc) as tc:
        with (
            tc.tile_pool(name="weights", bufs=1) as wpool,
            tc.tile_pool(name="acts", bufs=1) as apool,
            tc.tile_pool(name="pmm", bufs=4, space="PSUM") as pmm,
            tc.tile_pool(name="pacc", bufs=2, space="PSUM") as pacc,
            tc.tile_pool(name="pbc", bufs=2, space="PSUM") as pbc,
            tc.tile_pool(name="ptile", bufs=6) as ppool,
            tc.tile_pool(name="small", bufs=4) as spool,
            tc.tile_pool(name="ostage", bufs=4) as opool,
        ):
            # ---- load everything ----
            xT_s = [wpool.tile([128, T], BF16, tag=f"xT{i}", name=f"xT{i}") for i in range(NCT)]
            for i in range(NCT):
                nc.sync.dma_start(xT_s[i][:], xT[i * 128:(i + 1) * 128, :])
            wqkT_s = [wpool.tile([128, 2 * CH], BF16, tag=f"wqk{i}", name=f"wqk{i}") for i in range(NCT)]
            for i in range(NCT):
                nc.sync.dma_start(wqkT_s[i][:], wqkT[i * 128:(i + 1) * 128, :])
            wvT_s = [wpool.tile([128, CH], BF16, tag=f"wv{i}", name=f"wv{i}") for i in range(NCT)]
            for i in range(NCT):
                nc.sync.dma_start(wvT_s[i][:], wvT[i * 128:(i + 1) * 128, :])
            wpT_s = [wpool.tile([128, C], BF16, tag=f"wp{i}", name=f"wp{i}") for i in range(CH // 128)]
            for i in range(CH // 128):
                nc.sync.dma_start(wpT_s[i][:], wpT[i * 128:(i + 1) * 128, :])
            bqk_s = wpool.tile([128, 2 * CH // 128], F32, tag="bqk")
            nc.sync.dma_start(bqk_s[:], bqk[:, :])
            bv_s = wpool.tile([128, CH // 128], F32, tag="bv")
            nc.sync.dma_start(bv_s[:], bv[:, :])
            masks_s = wpool.tile([128, 4 * 512], BF16, tag="masks")
            nc.sync.dma_start(masks_s[:], masks[:, :])
            ones_s = wpool.tile([1, 64], F32, tag="ones")
            nc.vector.memset(ones_s[:], 1.0)

            # pre-touch DMA-loaded tensors on DVE so downstream DVE ops
            # (TensorScalarPtr supports only one embedded sync wait) don't
            # need to wait on both PE and DMA semaphores
            scratch = wpool.tile([128, 4], F32, tag="scratch")
            nc.vector.tensor_copy(scratch[:, 0:1], bqk_s[:, 0:1])
            nc.vector.tensor_copy(scratch[:, 1:2], bv_s[:, 0:1])
            nc.vector.tensor_copy(scratch[:, 2:3], masks_s[:, 0:1])

            # qkT rows: tiles 0..2 = Q^T (384 rows), 3..5 = K^T
            qkT_s = [apool.tile([128, T], BF16, tag=f"qkT{i}", name=f"qkT{i}") for i in range(NCT)]
            v_s = [apool.tile([128, HH * VW], BF16, tag=f"v{i}", name=f"v{i}") for i in range(NTT)]
            yT_s = [apool.tile([128, T], BF16, tag=f"yT{i}", name=f"yT{i}") for i in range(CH // 128)]

            # ---- QK^T projection: qkT[o, t] = W_qk @ x^T + b ----
            # emission order gets heads 0,1 (tiles 0 and 3) done first
            for ot in (0, 3, 1, 4, 2, 5):
                for tcn in range(NQC):
                    ps = pmm.tile([128, 512], F32, tag="mm")
                    for ct in range(NCT):
                        nc.tensor.matmul(
                            ps[:],
                            lhsT=wqkT_s[ct][:, ot * 128:(ot + 1) * 128],
                            rhs=xT_s[ct][:, tcn * 512:(tcn + 1) * 512],
                            start=(ct == 0),
                            stop=(ct == NCT - 1),
                        )
                    # bias add + cast to bf16 on DVE
                    nc.vector.tensor_scalar_add(
                        qkT_s[ot][:, tcn * 512:(tcn + 1) * 512], ps[:],
                        bqk_s[:, ot:ot + 1],
                    )

            # ---- V projection: v[t, h*65+d] = x @ Wv^T (no bias; ones col) ----
            for tt in range(NTT):
                ps = pmm.tile([128, CH], F32, tag="mm")
                for ct in range(NCT):
                    nc.tensor.matmul(
                        ps[:],
                        lhsT=xT_s[ct][:, tt * 128:(tt + 1) * 128],
                        rhs=wvT_s[ct][:],
                        start=(ct == 0),
                        stop=(ct == NCT - 1),
                    )
                v3 = v_s[tt][:].rearrange("p (h w) -> p h w", w=VW)
                nc.vector.tensor_copy(
                    v3[:, :, 0:64], ps[:].rearrange("p (h d) -> p h d", d=64)
                )
                nc.vector.memset(v3[:, :, 64:65], 1.0)

            # ---- attention, head pairs for PE row-group packing ----
            for hp in range(HH // 2):
                qt = hp          # Q^T rows for heads 2hp,2hp+1 live in tile hp
                ktile = 3 + hp   # K^T rows in tile 3+hp
                for qc in range(NQC):
                    o_acc = [pacc.tile([65, 512], F32, tag="oacc", name="oacc") for _ in range(2)]
                    nkt = 4 * (qc + 1)
                    for kt in range(nkt):
                        j = kt - 4 * qc
                        pts = []
                        for hi in range(2):
                            base = hi * 64
                            ss = pmm.tile([128, 512], F32, tag="mm")
                            nc.tensor.matmul(
                                ss[:],
                                lhsT=qkT_s[ktile][base:base + 64,
                                                  kt * 128:(kt + 1) * 128],
                                rhs=qkT_s[qt][base:base + 64,
                                              qc * 512:(qc + 1) * 512],
                                start=True, stop=True,
                            )
                            pt = ppool.tile([128, 512], BF16, tag="pt")
                            nc.scalar.activation(pt[:], ss[:], AF.Exp, scale=0.125)
                            if j >= 0:
                                nc.vector.tensor_mul(
                                    pt[:], pt[:], masks_s[:, j * 512:(j + 1) * 512]
                                )
                            pts.append(pt)
                        for hi in range(2):
                            h = 2 * hp + hi
                            nc.tensor.matmul(
                                o_acc[hi][:],
                                lhsT=v_s[kt][:, h * VW:(h + 1) * VW],
                                rhs=pts[hi][:],
                                start=(kt == 0),
                                stop=(kt == nkt - 1),
                            )
                    # normalize + v-bias -> y^T
                    for hi in range(2):
                        h = 2 * hp + hi
                        base = hi * 64
                        dn = spool.tile([1, 512], F32, tag="dn")
                        nc.scalar.copy(dn[:], o_acc[hi][64:65, :])
                        bc = pbc.tile([64, 512], F32, tag="bc")
                        nc.tensor.matmul(
                            bc[:],
                            lhsT=ones_s[:],
                            rhs=dn[:],
                            start=True, stop=True,
                        )
                        rc = spool.tile([64, 512], F32, tag="rc")
                        nc.vector.reciprocal(rc[:], bc[:])
                        ysl = yT_s[hp][base:base + 64, qc * 512:(qc + 1) * 512]
                        nc.vector.tensor_mul(ysl, o_acc[hi][0:64, :], rc[:])
                        nc.vector.tensor_scalar_add(
                            ysl, ysl, bv_s[base:base + 64, hp:hp + 1]
                        )

            # ---- c_proj (row-sharded, partial sums): out^T = Wp_half @ y^T ----
            for ot in range(NCT):
                for tcn in range(NQC):
                    ps = pmm.tile([128, 512], F32, tag="mm")
                    for ct in range(CH // 128):
                        nc.tensor.matmul(
                            ps[:],
                            lhsT=wpT_s[ct][:, ot * 128:(ot + 1) * 128],
                            rhs=yT_s[ct][:, tcn * 512:(tcn + 1) * 512],
                            start=(ct == 0),
                            stop=(ct == CH // 128 - 1),
                        )
                    so = opool.tile([128, 512], F32, tag="so")
                    nc.vector.tensor_copy(so[:], ps[:])
                    nc.sync.dma_start(
                        out[ot * 128:(ot + 1) * 128, tcn * 512:(tcn + 1) * 512],
                        so[:],
                    )
    nc.compile()
    return nc


_CACHE: dict = {}


def _get_graph():
    if "nc" not in _CACHE:
        _CACHE["nc"] = _build_graph()
    return _CACHE["nc"]


def _bf16(a):
    return np.ascontiguousarray(a.astype(ml_dtypes.bfloat16))


def _make_masks():
    k = np.arange(128)[:, None]
    q = np.arange(512)[None, :]
    m = np.zeros((128, 4 * 512), np.float32)
    for j in range(4):
        m[:, j * 512:(j + 1) * 512] = (q >= k + j * 128).astype(np.float32)
    return _bf16(m)


def _prepare_in_maps(x, W_attn, b_attn, W_proj):
    masks = _make_masks()
    in_maps = []
    for core in range(NCORES):
        b, hh = core // 2, core % 2
        sl = slice(hh * CH, (hh + 1) * CH)
        wq = W_attn[0 * C:1 * C][sl]          # [384, 768]
        wk = W_attn[1 * C:2 * C][sl]
        wv = W_attn[2 * C:3 * C][sl]
        bq = b_attn[0 * C:1 * C][sl]
        bk = b_attn[1 * C:2 * C][sl]
        bvv = b_attn[2 * C:3 * C][sl]
        in_maps.append({
            "xT": _bf16(x[b].T),                                   # [768, 2048]
            "wqkT": _bf16(np.concatenate([wq, wk], 0).T),          # [768, 768]
            "bqk": np.ascontiguousarray(
                np.concatenate([bq, bk]).reshape(-1, 128).T),      # [128, 6]
            "wvT": _bf16(wv.T),                                    # [768, 384]
            "bv": np.ascontiguousarray(bvv.reshape(-1, 128).T),    # [128, 3]
            "wpT": _bf16(W_proj[:, sl].T),                         # [384, 768]
            "masks": masks,
        })
    return in_maps


def _unshard(outs, b_proj):
    y = np.empty((B, T, C), np.float32)
    for b in range(B):
        y[b] = (outs[2 * b]["out"] + outs[2 * b + 1]["out"]).T + b_proj
    return y


def run(x, W_attn, b_attn, W_proj, b_proj, **spmd_kwargs):
    x = np.asarray(x, np.float32)
    W_attn = np.asarray(W_attn, np.float32)
    b_attn = np.asarray(b_attn, np.float32)
    W_proj = np.asarray(W_proj, np.float32)
    b_proj = np.asarray(b_proj, np.float32)
    in_maps = _prepare_in_maps(x, W_attn, b_attn, W_proj)
    nc = _get_graph()
    res = run_bass_kernel_spmd(
        nc, in_maps, core_ids=list(range(NCORES)), **spmd_kwargs
    )
    return _unshard(res.results, b_proj), res


def kernel(x, W_attn, b_attn, W_proj, b_proj):
    y, _ = run(x, W_attn, b_attn, W_proj, b_proj)
    return y


# revision 8
# speedup vs baseline: 1.4687x; 1.4687x over previous
"""Distributed causal multi-head attention block for 8 TRN2 NeuronCores.

Sharding: core i -> (batch b = i//2, head-half hh = i%2).  Each core computes
attention for 6 of the 12 heads of one batch element, then a row-sharded
c_proj (its 384 input channels -> full 768 outputs, partial sums).  The
host sums the two partial projections per batch (the "all-reduce" of the
tensor-parallel c_proj) and adds b_proj.

Everything on-chip lives transposed ([feature, token]) so no transposes are
needed:
  qkT = W_qk @ x^T          (heads' Q^T,K^T in [d, t] layout)
  V   = x @ Wv^T            ([t, d] layout, + per-head ones column)
  S^T = K_blk @ Q_blk^T     -> exp (scale 1/8 folded in) -> causal mask
  O^T_aug = [V|1]^T @ P^T   (row 64 of each head block = softmax denom)
  y^T = O^T * (1/denom)  + bv
  out^T = Wp_half @ y^T     (partial over this core's heads)
"""

import sys

sys.path.insert(0, "/opt/trn_rl_repo")

import numpy as np
import ml_dtypes

import concourse.bass as bass
import concourse.bacc as bacc
import concourse.mybir as mybir
import concourse.tile as tile
from concourse.bass_utils import run_bass_kernel_spmd

BF16 = mybir.dt.bfloat16
F32 = mybir.dt.float32
F32R = mybir.dt.float32r
AF = mybir.ActivationFunctionType
ALU = mybir.AluOpType

B, T, C, H, HD = 4, 2048, 768, 12, 64
NCORES = 8
HH = 6              # heads per core
CH = HH * HD        # 384 channels per core
NCT = C // 128      # 6 contraction tiles over C
NTT = T // 128      # 16 token tiles
NQC = T // 512      # 4 query chunks
VW = 65             # per-head V block width (64 dims + ones column)


def _build_graph():
    nc = bacc.Bacc("TRN2", target_bir_lowering=False)

    xT = nc.declare_dram_parameter("xT", [C, T], BF16, isOutput=False)
    wqkT = nc.declare_dram_parameter("wqkT", [C, 2 * CH], BF16, isOutput=False)
    bqk = nc.declare_dram_parameter("bqk", [128, 2 * CH // 128], F32, isOutput=False)
    wvT = nc.declare_dram_parameter("wvT", [C, CH], BF16, isOutput=False)
    bv = nc.declare_dram_parameter("bv", [128, CH // 128], F32, isOutput=False)
    wpT = nc.declare_dram_parameter("wpT", [CH, C], BF16, isOutput=False)
    masks = nc.declare_dram_parameter("masks", [128, 4 * 1024], BF16, isOutput=False)
    out = nc.declare_dram_parameter("out", [C, T], F32, isOutput=True)

    with tile.TileContext(nc) as tc:
        with (
            tc.tile_pool(name="weights", bufs=1) as wpool,
            tc.tile_pool(name="acts", bufs=1) as apool,
            tc.tile_pool(name="pmm", bufs=2, space="PSUM") as pmm,
            tc.tile_pool(name="ps2", bufs=2, space="PSUM") as ps2,
            tc.tile_pool(name="pacc", bufs=2, space="PSUM") as pacc,
            tc.tile_pool(name="ptile", bufs=4) as ppool,
            tc.tile_pool(name="small", bufs=4) as spool,
            tc.tile_pool(name="ostage", bufs=4) as opool,
        ):
            # ---- load everything ----
            xT_s = [wpool.tile([128, T], BF16, tag=f"xT{i}", name=f"xT{i}") for i in range(NCT)]
            for i in range(NCT):
                nc.sync.dma_start(xT_s[i][:], xT[i * 128:(i + 1) * 128, :])
            wqkT_s = [wpool.tile([128, 2 * CH], BF16, tag=f"wqk{i}", name=f"wqk{i}") for i in range(NCT)]
            for i in range(NCT):
                nc.sync.dma_start(wqkT_s[i][:], wqkT[i * 128:(i + 1) * 128, :])
            wvT_s = [wpool.tile([128, CH], BF16, tag=f"wv{i}", name=f"wv{i}") for i in range(NCT)]
            for i in range(NCT):
                nc.sync.dma_start(wvT_s[i][:], wvT[i * 128:(i + 1) * 128, :])
            wpT_s = [wpool.tile([128, C], BF16, tag=f"wp{i}", name=f"wp{i}") for i in range(CH // 128)]
            for i in range(CH // 128):
                nc.sync.dma_start(wpT_s[i][:], wpT[i * 128:(i + 1) * 128, :])
            bqk_s = wpool.tile([128, 2 * CH // 128], F32, tag="bqk")
            nc.sync.dma_start(bqk_s[:], bqk[:, :])
            bv_s = wpool.tile([128, CH // 128], F32, tag="bv")
            nc.sync.dma_start(bv_s[:], bv[:, :])
            masks_s = wpool.tile([128, 4 * 1024], BF16, tag="masks")
            nc.sync.dma_start(masks_s[:], masks[:, :])
            # pre-touch DMA-loaded tensors on DVE so downstream DVE ops
            # (TensorScalarPtr supports only one embedded sync wait) don't
            # need to wait on both PE and DMA semaphores
            scratch = wpool.tile([128, 4], F32, tag="scratch")
            nc.vector.tensor_copy(scratch[:, 0:1], bqk_s[:, 0:1])
            nc.vector.tensor_copy(scratch[:, 1:2], bv_s[:, 0:1])
            nc.vector.tensor_copy(scratch[:, 2:3], masks_s[:, 0:1])

            # qkT rows: tiles 0..2 = Q^T (384 rows), 3..5 = K^T
            qkT_s = [apool.tile([128, T], BF16, tag=f"qkT{i}", name=f"qkT{i}") for i in range(NCT)]
            v_s = [apool.tile([128, HH * VW], BF16, tag=f"v{i}", name=f"v{i}") for i in range(NTT)]
            yT_s = [apool.tile([128, T], BF16, tag=f"yT{i}", name=f"yT{i}") for i in range(CH // 128)]

            # ---- QK^T projection: qkT[o, t] = W_qk @ x^T + b ----
            # emission order gets heads 0,1 (tiles 0 and 3) done first
            for ot in (0, 3, 1, 4, 2, 5):
                for tcn in range(NQC):
                    ps = pmm.tile([128, 512], F32, tag="mm")
                    for ct in range(NCT):
                        nc.tensor.matmul(
                            ps[:],
                            lhsT=wqkT_s[ct][:, ot * 128:(ot + 1) * 128],
                            rhs=xT_s[ct][:, tcn * 512:(tcn + 1) * 512],
                            start=(ct == 0),
                            stop=(ct == NCT - 1),
                        )
                    # bias add + cast to bf16 on DVE
                    nc.vector.tensor_scalar_add(
                        qkT_s[ot][:, tcn * 512:(tcn + 1) * 512], ps[:],
                        bqk_s[:, ot:ot + 1],
                    )

            # ---- V projection: v[t, h*65+d] = x @ Wv^T (no bias; ones col) ----
            for tt in range(NTT):
                ps = pmm.tile([128, CH], F32, tag="mm")
                for ct in range(NCT):
                    nc.tensor.matmul(
                        ps[:],
                        lhsT=xT_s[ct][:, tt * 128:(tt + 1) * 128],
                        rhs=wvT_s[ct][:],
                        start=(ct == 0),
                        stop=(ct == NCT - 1),
                    )
                v3 = v_s[tt][:].rearrange("p (h w) -> p h w", w=VW)
                nc.vector.tensor_copy(
                    v3[:, :, 0:64], ps[:].rearrange("p (h d) -> p h d", d=64)
                )
                nc.vector.memset(v3[:, :, 64:65], 1.0)

            # ---- attention, head pairs for PE row-group packing ----
            # S^T of both heads land side by side in one 2-bank PSUM tile so
            # a single exp covers both (amortizes ACT's 352-cycle pipe fill)
            for hp in range(HH // 2):
                qt = hp          # Q^T rows for heads 2hp,2hp+1 live in tile hp
                ktile = 3 + hp   # K^T rows in tile 3+hp
                for qc in range(NQC):
                    o_acc = [pacc.tile([65, 512], F32, tag="oacc", name="oacc")
                             for _ in range(2)]
                    nkt = 4 * (qc + 1)
                    for kt in range(nkt):
                        j = kt - 4 * qc
                        s2 = ps2.tile([128, 1024], F32, tag="s2")
                        for hi in range(2):
                            base = hi * 64
                            nc.tensor.matmul(
                                s2[:, hi * 512:(hi + 1) * 512],
                                lhsT=qkT_s[ktile][base:base + 64,
                                                  kt * 128:(kt + 1) * 128],
                                rhs=qkT_s[qt][base:base + 64,
                                              qc * 512:(qc + 1) * 512],
                                start=True, stop=True,
                            )
                        p2 = ppool.tile([128, 1024], BF16, tag="pt")
                        nc.scalar.activation(p2[:], s2[:], AF.Exp, scale=0.125)
                        if j >= 0:
                            nc.vector.tensor_mul(
                                p2[:], p2[:], masks_s[:, j * 1024:(j + 1) * 1024]
                            )
                        for hi in range(2):
                            h = 2 * hp + hi
                            nc.tensor.matmul(
                                o_acc[hi][:],
                                lhsT=v_s[kt][:, h * VW:(h + 1) * VW],
                                rhs=p2[:, hi * 512:(hi + 1) * 512],
                                start=(kt == 0),
                                stop=(kt == nkt - 1),
                            )
                    # normalize + v-bias -> y^T
                    for hi in range(2):
                        h = 2 * hp + hi
                        base = hi * 64
                        dn = spool.tile([1, 512], F32, tag="dn")
                        nc.vector.tensor_copy(dn[:], o_acc[hi][64:65, :])
                        rn = spool.tile([1, 512], F32, tag="rn")
                        nc.vector.reciprocal_approx_fast(rn[:], dn[:])
                        rc = spool.tile([64, 512], F32, tag="rc")
                        nc.gpsimd.partition_broadcast(rc[:], rn[:], channels=64)
                        ysl = yT_s[hp][base:base + 64, qc * 512:(qc + 1) * 512]
                        nc.vector.tensor_mul(ysl, o_acc[hi][0:64, :], rc[:])
                        nc.vector.tensor_scalar_add(
                            ysl, ysl, bv_s[base:base + 64, hp:hp + 1]
                        )

            # ---- c_proj (row-sharded, partial sums): out^T = Wp_half @ y^T ----
            for ot in range(NCT):
                for tcn in range(NQC):
                    ps = pmm.tile([128, 512], F32, tag="mm")
                    for ct in range(CH // 128):
                        nc.tensor.matmul(
                            ps[:],
                            lhsT=wpT_s[ct][:, ot * 128:(ot + 1) * 128],
                            rhs=yT_s[ct][:, tcn * 512:(tcn + 1) * 512],
                            start=(ct == 0),
                            stop=(ct == CH // 128 - 1),
                        )
                    so = opool.tile([128, 512], F32, tag="so")
                    nc.vector.tensor_copy(so[:], ps[:])
                    nc.sync.dma_start(
                        out[ot * 128:(ot + 1) * 128, tcn * 512:(tcn + 1) * 512],
                        so[:],
                    )
    nc.compile()
    return nc


_CACHE: dict = {}


def _get_graph():
    if "nc" not in _CACHE:
        _CACHE["nc"] = _build_graph()
    return _CACHE["nc"]


def _bf16(a):
    return np.ascontiguousarray(a.astype(ml_dtypes.bfloat16))


def _make_masks():
    k = np.arange(128)[:, None]
    q = np.arange(512)[None, :]
    m = np.zeros((128, 4 * 1024), np.float32)
    for j in range(4):
        pat = (q >= k + j * 128).astype(np.float32)
        m[:, j * 1024:j * 1024 + 512] = pat
        m[:, j * 1024 + 512:(j + 1) * 1024] = pat
    return _bf16(m)


def _prepare_in_maps(x, W_attn, b_attn, W_proj):
    masks = _make_masks()
    in_maps = []
    for core in range(NCORES):
        b, hh = core // 2, core % 2
        sl = slice(hh * CH, (hh + 1) * CH)
        wq = W_attn[0 * C:1 * C][sl]          # [384, 768]
        wk = W_attn[1 * C:2 * C][sl]
        wv = W_attn[2 * C:3 * C][sl]
        bq = b_attn[0 * C:1 * C][sl]
        bk = b_attn[1 * C:2 * C][sl]
        bvv = b_attn[2 * C:3 * C][sl]
        in_maps.append({
            "xT": _bf16(x[b].T),                                   # [768, 2048]
            "wqkT": _bf16(np.concatenate([wq, wk], 0).T),          # [768, 768]
            "bqk": np.ascontiguousarray(
                np.concatenate([bq, bk]).reshape(-1, 128).T),      # [128, 6]
            "wvT": _bf16(wv.T),                                    # [768, 384]
            "bv": np.ascontiguousarray(bvv.reshape(-1, 128).T),    # [128, 3]
            "wpT": _bf16(W_proj[:, sl].T),                         # [384, 768]
            "masks": masks,
        })
    return in_maps


def _unshard(outs, b_proj):
    y = np.empty((B, T, C), np.float32)
    for b in range(B):
        y[b] = (outs[2 * b]["out"] + outs[2 * b + 1]["out"]).T + b_proj
    return y


def run(x, W_attn, b_attn, W_proj, b_proj, **spmd_kwargs):
    x = np.asarray(x, np.float32)
    W_attn = np.asarray(W_attn, np.float32)
    b_attn = np.asarray(b_attn, np.float32)
    W_proj = np.asarray(W_proj, np.float32)
    b_proj = np.asarray(b_proj, np.float32)
    in_maps = _prepare_in_maps(x, W_attn, b_attn, W_proj)
    nc = _get_graph()
    res = run_bass_kernel_spmd(
        nc, in_maps, core_ids=list(range(NCORES)), **spmd_kwargs
    )
    return _unshard(res.results, b_proj), res


def kernel(x, W_attn, b_attn, W_proj, b_proj):
    y, _ = run(x, W_attn, b_attn, W_proj, b_proj)
    return y


# revision 10
# speedup vs baseline: 1.5440x; 1.0513x over previous
"""Distributed causal multi-head attention block for 8 TRN2 NeuronCores.

Sharding: core i -> (batch b = i//2, head-half hh = i%2).  Each core computes
attention for 6 of the 12 heads of one batch element, then a row-sharded
c_proj (its 384 input channels -> full 768 outputs, partial sums).  The
host sums the two partial projections per batch (the "all-reduce" of the
tensor-parallel c_proj) and adds b_proj.

Everything on-chip lives transposed ([feature, token]) so no transposes are
needed:
  qkT = W_qk @ x^T          (heads' Q^T,K^T in [d, t] layout)
  V   = x @ Wv^T            ([t, d] layout, + per-head ones column)
  S^T = K_blk @ Q_blk^T     -> exp (scale 1/8 folded in) -> causal mask
  O^T_aug = [V|1]^T @ P^T   (row 64 of each head block = softmax denom)
  y^T = O^T * (1/denom)  + bv
  out^T = Wp_half @ y^T     (partial over this core's heads)
"""

import sys

sys.path.insert(0, "/opt/trn_rl_repo")

import numpy as np
import ml_dtypes

import concourse.bass as bass
import concourse.bacc as bacc
import concourse.mybir as mybir
import concourse.tile as tile
from concourse.bass_utils import run_bass_kernel_spmd

BF16 = mybir.dt.bfloat16
F32 = mybir.dt.float32
F32R = mybir.dt.float32r
AF = mybir.ActivationFunctionType
ALU = mybir.AluOpType

B, T, C, H, HD = 4, 2048, 768, 12, 64
NCORES = 8
HH = 6              # heads per core
CH = HH * HD        # 384 channels per core
NCT = C // 128      # 6 contraction tiles over C
NTT = T // 128      # 16 token tiles
NQC = T // 512      # 4 query chunks
VW = 65             # per-head V block width (64 dims + ones column)


def _build_graph():
    nc = bacc.Bacc("TRN2", target_bir_lowering=False)

    xT = nc.declare_dram_parameter("xT", [C, T], BF16, isOutput=False)
    wqkT = nc.declare_dram_parameter("wqkT", [C, 2 * CH], BF16, isOutput=False)
    bqk = nc.declare_dram_parameter("bqk", [128, 2 * CH // 128], F32, isOutput=False)
    wvT = nc.declare_dram_parameter("wvT", [C, CH], BF16, isOutput=False)
    bv = nc.declare_dram_parameter("bv", [128, CH // 128], F32, isOutput=False)
    wpT = nc.declare_dram_parameter("wpT", [CH, C], BF16, isOutput=False)
    masks = nc.declare_dram_parameter("masks", [128, 4 * 1024], BF16, isOutput=False)
    out = nc.declare_dram_parameter("out", [C, T], F32, isOutput=True)

    with tile.TileContext(nc) as tc:
        with (
            tc.tile_pool(name="weights", bufs=1) as wpool,
            tc.tile_pool(name="acts", bufs=1) as apool,
            tc.tile_pool(name="ps2", bufs=2, space="PSUM") as ps2,
            tc.tile_pool(name="pacc", bufs=2, space="PSUM") as pacc,
            tc.tile_pool(name="ptile", bufs=4) as ppool,
            tc.tile_pool(name="small", bufs=4) as spool,
            tc.tile_pool(name="ostage", bufs=4) as opool,
        ):
            # ---- load everything ----
            xT_s = [wpool.tile([128, T], BF16, tag=f"xT{i}", name=f"xT{i}") for i in range(NCT)]
            for i in range(NCT):
                nc.sync.dma_start(xT_s[i][:], xT[i * 128:(i + 1) * 128, :])
            wqkT_s = [wpool.tile([128, 2 * CH], BF16, tag=f"wqk{i}", name=f"wqk{i}") for i in range(NCT)]
            for i in range(NCT):
                nc.sync.dma_start(wqkT_s[i][:], wqkT[i * 128:(i + 1) * 128, :])
            wvT_s = [wpool.tile([128, CH], BF16, tag=f"wv{i}", name=f"wv{i}") for i in range(NCT)]
            for i in range(NCT):
                nc.sync.dma_start(wvT_s[i][:], wvT[i * 128:(i + 1) * 128, :])
            wpT_s = [wpool.tile([128, C], BF16, tag=f"wp{i}", name=f"wp{i}") for i in range(CH // 128)]
            for i in range(CH // 128):
                nc.sync.dma_start(wpT_s[i][:], wpT[i * 128:(i + 1) * 128, :])
            bqk_s = wpool.tile([128, 2 * CH // 128], F32, tag="bqk")
            nc.sync.dma_start(bqk_s[:], bqk[:, :])
            bv_s = wpool.tile([128, CH // 128], F32, tag="bv")
            nc.sync.dma_start(bv_s[:], bv[:, :])
            masks_s = wpool.tile([128, 4 * 1024], BF16, tag="masks")
            nc.sync.dma_start(masks_s[:], masks[:, :])
            # pre-touch DMA-loaded tensors on DVE so downstream DVE ops
            # (TensorScalarPtr supports only one embedded sync wait) don't
            # need to wait on both PE and DMA semaphores
            scratch = wpool.tile([128, 4], F32, tag="scratch")
            nc.vector.tensor_copy(scratch[:, 0:1], bqk_s[:, 0:1])
            nc.vector.tensor_copy(scratch[:, 1:2], bv_s[:, 0:1])
            nc.vector.tensor_copy(scratch[:, 2:3], masks_s[:, 0:1])

            # qkT rows: tiles 0..2 = Q^T (384 rows), 3..5 = K^T
            qkT_s = [apool.tile([128, T], BF16, tag=f"qkT{i}", name=f"qkT{i}") for i in range(NCT)]
            v_s = [apool.tile([128, HH * VW], BF16, tag=f"v{i}", name=f"v{i}") for i in range(NTT)]
            yT_s = [apool.tile([128, T], BF16, tag=f"yT{i}", name=f"yT{i}") for i in range(CH // 128)]

            # ---- QK^T projection: qkT[o, t] = W_qk @ x^T + b ----
            # emission order gets heads 0,1 (tiles 0 and 3) done first
            for ot in (0, 3, 1, 4, 2, 5):
                for tcn in range(NQC):
                    ps = ps2.tile([128, 512], F32, tag="mm")
                    for ct in range(NCT):
                        nc.tensor.matmul(
                            ps[:],
                            lhsT=wqkT_s[ct][:, ot * 128:(ot + 1) * 128],
                            rhs=xT_s[ct][:, tcn * 512:(tcn + 1) * 512],
                            start=(ct == 0),
                            stop=(ct == NCT - 1),
                        )
                    # bias add + cast to bf16 on DVE
                    nc.vector.tensor_scalar_add(
                        qkT_s[ot][:, tcn * 512:(tcn + 1) * 512], ps[:],
                        bqk_s[:, ot:ot + 1],
                    )

            # ---- V projection: v[t, h*65+d] = x @ Wv^T (no bias; ones col) ----
            for tt in range(NTT):
                ps = ps2.tile([128, CH], F32, tag="mm")
                for ct in range(NCT):
                    nc.tensor.matmul(
                        ps[:],
                        lhsT=xT_s[ct][:, tt * 128:(tt + 1) * 128],
                        rhs=wvT_s[ct][:],
                        start=(ct == 0),
                        stop=(ct == NCT - 1),
                    )
                v3 = v_s[tt][:].rearrange("p (h w) -> p h w", w=VW)
                nc.vector.tensor_copy(
                    v3[:, :, 0:64], ps[:].rearrange("p (h d) -> p h d", d=64)
                )
                nc.vector.memset(v3[:, :, 64:65], 1.0)

            # ---- attention, head pairs for PE row-group packing ----
            # S^T of both heads land side by side in one 2-bank PSUM tile so
            # a single exp covers both (amortizes ACT's 352-cycle pipe fill)
            for hp in range(HH // 2):
                qt = hp          # Q^T rows for heads 2hp,2hp+1 live in tile hp
                ktile = 3 + hp   # K^T rows in tile 3+hp
                for qc in range(NQC):
                    o_acc = [pacc.tile([65, 512], F32, tag="oacc", name="oacc")
                             for _ in range(2)]
                    nkt = 4 * (qc + 1)
                    for kt in range(nkt):
                        j = kt - 4 * qc
                        s2 = ps2.tile([128, 1024], F32, tag="s2")
                        for hi in range(2):
                            base = hi * 64
                            nc.tensor.matmul(
                                s2[:, hi * 512:(hi + 1) * 512],
                                lhsT=qkT_s[ktile][base:base + 64,
                                                  kt * 128:(kt + 1) * 128],
                                rhs=qkT_s[qt][base:base + 64,
                                              qc * 512:(qc + 1) * 512],
                                start=True, stop=True,
                            )
                        p2 = ppool.tile([128, 1024], BF16, tag="pt")
                        nc.scalar.activation(p2[:], s2[:], AF.Exp, scale=0.125)
                        if j >= 0:
                            nc.vector.tensor_mul(
                                p2[:], p2[:], masks_s[:, j * 1024:(j + 1) * 1024]
                            )
                        for hi in range(2):
                            h = 2 * hp + hi
                            nc.tensor.matmul(
                                o_acc[hi][:],
                                lhsT=v_s[kt][:, h * VW:(h + 1) * VW],
                                rhs=p2[:, hi * 512:(hi + 1) * 512],
                                start=(kt == 0),
                                stop=(kt == nkt - 1),
                            )
                    # normalize + v-bias -> y^T.  First copy the whole
                    # accumulator to SBUF (single DVE op) so the PSUM bank is
                    # released immediately; normalize from the copy.
                    for hi in range(2):
                        h = 2 * hp + hi
                        base = hi * 64
                        dn = spool.tile([1, 512], F32, tag="dn")
                        nc.vector.tensor_copy(dn[:], o_acc[hi][64:65, :])
                        ob = spool.tile([64, 512], F32, tag="ob")
                        nc.vector.tensor_copy(ob[:], o_acc[hi][0:64, :])
                        rn = spool.tile([1, 512], F32, tag="rn")
                        nc.vector.reciprocal_approx_fast(rn[:], dn[:])
                        rc = spool.tile([64, 512], F32, tag="rc")
                        nc.gpsimd.partition_broadcast(rc[:], rn[:], channels=64)
                        ysl = yT_s[hp][base:base + 64, qc * 512:(qc + 1) * 512]
                        nc.vector.tensor_mul(ysl, ob[:], rc[:])
                        nc.vector.tensor_scalar_add(
                            ysl, ysl, bv_s[base:base + 64, hp:hp + 1]
                        )

            # ---- c_proj (row-sharded, partial sums): out^T = Wp_half @ y^T ----
            for ot in range(NCT):
                for tcn in range(NQC):
                    ps = ps2.tile([128, 512], F32, tag="mm")
                    for ct in range(CH // 128):
                        nc.tensor.matmul(
                            ps[:],
                            lhsT=wpT_s[ct][:, ot * 128:(ot + 1) * 128],
                            rhs=yT_s[ct][:, tcn * 512:(tcn + 1) * 512],
                            start=(ct == 0),
                            stop=(ct == CH // 128 - 1),
                        )
                    so = opool.tile([128, 512], F32, tag="so")
                    nc.vector.tensor_copy(so[:], ps[:])
                    nc.sync.dma_start(
                        out[ot * 128:(ot + 1) * 128, tcn * 512:(tcn + 1) * 512],
                        so[:],
                    )
    nc.compile()
    return nc


_CACHE: dict = {}


def _get_graph():
    if "nc" not in _CACHE:
        _CACHE["nc"] = _build_graph()
    return _CACHE["nc"]


def _bf16(a):
    return np.ascontiguousarray(a.astype(ml_dtypes.bfloat16))


def _make_masks():
    k = np.arange(128)[:, None]
    q = np.arange(512)[None, :]
    m = np.zeros((128, 4 * 1024), np.float32)
    for j in range(4):
        pat = (q >= k + j * 128).astype(np.float32)
        m[:, j * 1024:j * 1024 + 512] = pat
        m[:, j * 1024 + 512:(j + 1) * 1024] = pat
    return _bf16(m)


def _prepare_in_maps(x, W_attn, b_attn, W_proj):
    masks = _make_masks()
    in_maps = []
    for core in range(NCORES):
        b, hh = core // 2, core % 2
        sl = slice(hh * CH, (hh + 1) * CH)
        wq = W_attn[0 * C:1 * C][sl]          # [384, 768]
        wk = W_attn[1 * C:2 * C][sl]
        wv = W_attn[2 * C:3 * C][sl]
        bq = b_attn[0 * C:1 * C][sl]
        bk = b_attn[1 * C:2 * C][sl]
        bvv = b_attn[2 * C:3 * C][sl]
        in_maps.append({
            "xT": _bf16(x[b].T),                                   # [768, 2048]
            "wqkT": _bf16(np.concatenate([wq, wk], 0).T),          # [768, 768]
            "bqk": np.ascontiguousarray(
                np.concatenate([bq, bk]).reshape(-1, 128).T),      # [128, 6]
            "wvT": _bf16(wv.T),                                    # [768, 384]
            "bv": np.ascontiguousarray(bvv.reshape(-1, 128).T),    # [128, 3]
            "wpT": _bf16(W_proj[:, sl].T),                         # [384, 768]
            "masks": masks,
        })
    return in_maps


def _unshard(outs, b_proj):
    y = np.empty((B, T, C), np.float32)
    for b in range(B):
        y[b] = (outs[2 * b]["out"] + outs[2 * b + 1]["out"]).T + b_proj
    return y


def run(x, W_attn, b_attn, W_proj, b_proj, **spmd_kwargs):
    x = np.asarray(x, np.float32)
    W_attn = np.asarray(W_attn, np.float32)
    b_attn = np.asarray(b_attn, np.float32)
    W_proj = np.asarray(W_proj, np.float32)
    b_proj = np.asarray(b_proj, np.float32)
    in_maps = _prepare_in_maps(x, W_attn, b_attn, W_proj)
    nc = _get_graph()
    res = run_bass_kernel_spmd(
        nc, in_maps, core_ids=list(range(NCORES)), **spmd_kwargs
    )
    return _unshard(res.results, b_proj), res


def kernel(x, W_attn, b_attn, W_proj, b_proj):
    y, _ = run(x, W_attn, b_attn, W_proj, b_proj)
    return y
